# revision 8
# baseline (speedup 1.0000x reference)
"""DPGraphConvolution on 8 Trainium2 NeuronCores.

Computes out[b] = adj[b] @ (text[b] @ W) + bias for b = 0..7, one batch
element per core (data-parallel over batch, per the sharding hint).

Per-core algorithm (L=4096, F=64):
  The TensorE contracts over the partition dim, so adj must be presented
  with its column index j on partitions. Instead of transposing adj on
  chip, we exploit that the contraction is an order-invariant sum:

  * As part of host-side sharding prep, adj[b] is re-blocked so that
    SBUF partition p holds columns j = 32*p + u (u = 0..31) -- the
    device then streams fully contiguous 4 MB tiles at HBM line rate.
  * hidden = text @ W is computed on-chip with its rows stored in the
    matching order: hid[p, u*64+o] = hidden[32p+u, o], which is just
    hidden.reshape(128, 32*64) row-major.
  * For each output subtile (256 rows of out), 32 accumulating matmuls
    (one per u) compute outT[o, i] = sum_j hidden[j, o] * adj[i, j]
    exactly, using float32r at moving-dim 256 (full PE rate).

  The kernel writes out^T ([64, 4096]) per core; the host transposes.
"""

import numpy as np

import concourse.bass as bass
import concourse.mybir as mybir
import concourse.tile as tile
from concourse import bacc
from concourse.bass_utils import run_bass_kernel_spmd
from concourse.masks import make_identity

f32 = mybir.dt.float32
f32r = mybir.dt.float32r

B = 8
L, F = 4096, 64
P, U = 128, 32          # j = U*p + u ; requires P*U == L
NSUB = 256              # output rows (moving free dim) per matmul group
NT = L // NSUB          # 16 subtiles


def build_nc(reps: int = 1, nsub: int = NSUB, xbufs: int = 4):
    """Build the per-core Bass program. `reps` repeats the main loop for
    timing measurements (outputs are overwritten idempotently)."""
    nt = L // nsub
    nc = bacc.Bacc("TRN2", target_bir_lowering=False)
    text_d = nc.dram_tensor("text", [L, F], f32, kind="ExternalInput")
    # adj, host-re-blocked: adj_il[s, p, n, u] = adj[s*NSUB + n, U*p + u].
    # Declared fp32r (same bits as fp32) to satisfy the fp32r-matmul
    # producer check; HW uses the top 20 bits (s/e8/m11).
    adj_d = nc.dram_tensor("adj_il", [nt, P, nsub, U], f32r, kind="ExternalInput")
    w_d = nc.dram_tensor("weight", [F, F], f32, kind="ExternalInput")
    b_d = nc.dram_tensor("bias", [F], f32, kind="ExternalInput")
    out_d = nc.dram_tensor("out_t", [F, L], f32, kind="ExternalOutput")

    with tile.TileContext(nc) as tc:
        with tc.tile_pool(name="const", bufs=1) as cpool, \
             tc.tile_pool(name="xp", bufs=xbufs) as xpool, \
             tc.tile_pool(name="sm", bufs=3) as spool, \
             tc.tile_pool(name="pmain", bufs=3, space="PSUM") as pmain, \
             tc.tile_pool(name="pprep", bufs=2, space="PSUM") as pprep:

            ident = cpool.tile([P, P], f32)
            make_identity(nc, ident[:])
            w_sb = cpool.tile([P, F], f32)           # rows [:F] used
            nc.scalar.dma_start(w_sb[:F, :], w_d[:])
            bias_sb = cpool.tile([P, 1], f32)
            nc.scalar.dma_start(bias_sb[:F, :], b_d[:].rearrange("(f o) -> f o", o=1))

            # textT[f, j] on partitions 0..63, j contiguous in free dim.
            textT = cpool.tile([P, L], f32)
            text_v = text_d[:].rearrange("(r p) f -> r p f", p=P)
            RB = 4                                   # transposes per PSUM bank
            for rg in range(L // P // RB):
                pt = pprep.tile([F, RB * P], f32, tag="pt")
                for rr in range(RB):
                    r = rg * RB + rr
                    tn = spool.tile([P, F], f32, tag="tn")
                    nc.scalar.dma_start(tn[:], text_v[r])
                    nc.tensor.transpose(pt[:, rr * P:(rr + 1) * P], tn[:], ident[:])
                nc.vector.tensor_copy(textT[:F, rg * RB * P:(rg + 1) * RB * P], pt[:])

            # hid[p, u*F + o] = hidden[U*p + u, o] = (text @ W)[U*p+u, o]
            # Stored as fp32r: the PSUM->SBUF copy performs the rounding.
            hid = cpool.tile([P, U * F], f32r)
            textT3 = textT[:F, :].rearrange("f (p u) -> f p u", u=U)
            HG = 8                                   # matmuls per PSUM bank
            for g in range(U // HG):
                ph = pprep.tile([P, HG * F], f32, tag="ph")
                for uu in range(HG):
                    u = g * HG + uu
                    nc.tensor.matmul(
                        ph[:, uu * F:(uu + 1) * F],
                        lhsT=textT3[:, :, u],
                        rhs=w_sb[:F, :],
                        start=True, stop=True,
                    )
                nc.vector.tensor_copy(hid[:, g * HG * F:(g + 1) * HG * F], ph[:])

            hid3 = hid[:].rearrange("p (u f) -> p u f", u=U)

            for rep in range(reps):
                for s in range(nt):
                    x = xpool.tile([P, nsub, U], f32r, tag="x")
                    nc.sync.dma_start(x[:], adj_d[s])
                    po = pmain.tile([F, nsub], f32, tag="po")
                    for u in range(U):
                        nc.tensor.matmul(
                            po[:],
                            lhsT=hid3[:, u, :],
                            rhs=x[:, :, u],
                            start=(u == 0), stop=(u == U - 1),
                        )
                    ot = spool.tile([F, nsub], f32, tag="ot")
                    nc.vector.tensor_scalar_add(ot[:], po[:], bias_sb[:F, :])
                    nc.scalar.dma_start(out_d[:, s * nsub:(s + 1) * nsub], ot[:])

    nc.finalize()
    return nc


def interleave_adj(adj, nsub: int = NSUB):
    """Host-side sharding prep: re-block adj so each core's DMA is fully
    contiguous.  adj [B, L, L] -> [B, L//nsub, P, nsub, U] with
    adj_il[b, s, p, n, u] = adj[b, s*nsub + n, U*p + u]."""
    from concurrent.futures import ThreadPoolExecutor
    nt = L // nsub
    out = np.empty((B, nt, P, nsub, U), dtype=np.float32)
    src = adj.reshape(B, nt, nsub, P, U)

    def one(b):
        np.copyto(out[b], src[b].transpose(0, 2, 1, 3))

    with ThreadPoolExecutor(max_workers=B) as ex:
        list(ex.map(one, range(B)))
    return out


_NC_CACHE = None


def kernel(text, adj, weight, bias):
    global _NC_CACHE
    text = np.ascontiguousarray(np.asarray(text, dtype=np.float32))
    adj = np.asarray(adj, dtype=np.float32)
    weight = np.ascontiguousarray(np.asarray(weight, dtype=np.float32))
    bias = np.ascontiguousarray(np.asarray(bias, dtype=np.float32))
    assert text.shape == (B, L, F) and adj.shape == (B, L, L)

    adj_il = interleave_adj(adj)

    if _NC_CACHE is None:
        _NC_CACHE = build_nc()
    nc = _NC_CACHE

    in_maps = [
        {"text": text[b], "adj_il": adj_il[b], "weight": weight, "bias": bias}
        for b in range(B)
    ]
    res = run_bass_kernel_spmd(nc, in_maps, list(range(B)))
    out = np.stack([res.results[b]["out_t"].T for b in range(B)], axis=0)
    return np.ascontiguousarray(out, dtype=np.float32)


# revision 11
# speedup vs baseline: 1.3971x; 1.3971x over previous
"""DPGraphConvolution on 8 Trainium2 NeuronCores.

Computes out[b] = adj[b] @ (text[b] @ W) + bias for b = 0..7, one batch
element per core (data-parallel over batch, per the sharding hint).

Per-core algorithm (L=4096, F=64):
  The TensorE contracts over the partition dim, so adj must be presented
  with its column index j on partitions. Instead of transposing adj on
  chip, we exploit that the contraction is an order-invariant sum:

  * As part of host-side sharding prep, adj[b] is re-blocked so that
    SBUF partition p holds columns j = 32*p + u (u = 0..31) -- the
    device then streams fully contiguous 4 MB tiles at HBM line rate.
  * hidden = text @ W is computed on-chip with its rows stored in the
    matching order: hid[p, u*64+o] = hidden[32p+u, o], which is just
    hidden.reshape(128, 32*64) row-major.
  * For each output subtile (256 rows of out), 32 accumulating matmuls
    (one per u) compute outT[o, i] = sum_j hidden[j, o] * adj[i, j]
    exactly, using float32r at moving-dim 256 (full PE rate).

  The kernel writes out^T ([64, 4096]) per core; the host transposes.
"""

import numpy as np

import concourse.bass as bass
import concourse.mybir as mybir
import concourse.tile as tile
from concourse import bacc
from concourse.bass_utils import run_bass_kernel_spmd
from concourse.masks import make_identity

f32 = mybir.dt.float32
f32r = mybir.dt.float32r

B = 8
L, F = 4096, 64
P, U = 128, 32          # j = U*p + u ; requires P*U == L
NSUB = 256              # output rows (moving free dim) per matmul group
NT = L // NSUB          # 16 subtiles


def build_nc(reps: int = 1, nsub: int = NSUB, xbufs: int = 4,
             small_on_scalar: bool = True, x_split: bool = False,
             small_eng: str | None = None):
    """Build the per-core Bass program. `reps` repeats the main loop for
    timing measurements (outputs are overwritten idempotently)."""
    nt = L // nsub
    nc = bacc.Bacc("TRN2", target_bir_lowering=False)
    sm_eng_name = small_eng or ("scalar" if small_on_scalar else "sync")
    text_d = nc.dram_tensor("text", [L, F], f32, kind="ExternalInput")
    # adj, host-re-blocked: adj_il[s, p, n, u] = adj[s*NSUB + n, U*p + u].
    # Declared fp32r (same bits as fp32) to satisfy the fp32r-matmul
    # producer check; HW uses the top 20 bits (s/e8/m11).
    adj_d = nc.dram_tensor("adj_il", [nt, P, nsub, U], f32r, kind="ExternalInput")
    w_d = nc.dram_tensor("weight", [F, F], f32, kind="ExternalInput")
    b_d = nc.dram_tensor("bias", [F], f32, kind="ExternalInput")
    out_d = nc.dram_tensor("out_t", [F, L], f32, kind="ExternalOutput")

    with tile.TileContext(nc) as tc:
        with tc.tile_pool(name="const", bufs=1) as cpool, \
             tc.tile_pool(name="xp", bufs=xbufs) as xpool, \
             tc.tile_pool(name="sm", bufs=3) as spool, \
             tc.tile_pool(name="pmain", bufs=3, space="PSUM") as pmain, \
             tc.tile_pool(name="pprep", bufs=2, space="PSUM") as pprep:

            ident = cpool.tile([P, P], f32)
            make_identity(nc, ident[:])
            w_sb = cpool.tile([P, F], f32)           # rows [:F] used
            getattr(nc, sm_eng_name).dma_start(w_sb[:F, :], w_d[:])
            bias_sb = cpool.tile([P, 1], f32)
            getattr(nc, sm_eng_name).dma_start(bias_sb[:F, :], b_d[:].rearrange("(f o) -> f o", o=1))

            # textT[f, j] on partitions 0..63, j contiguous in free dim.
            textT = cpool.tile([P, L], f32)
            text_v = text_d[:].rearrange("(r p) f -> r p f", p=P)
            RB = 4                                   # transposes per PSUM bank
            for rg in range(L // P // RB):
                pt = pprep.tile([F, RB * P], f32, tag="pt")
                for rr in range(RB):
                    r = rg * RB + rr
                    tn = spool.tile([P, F], f32, tag="tn")
                    getattr(nc, sm_eng_name).dma_start(tn[:], text_v[r])
                    nc.tensor.transpose(pt[:, rr * P:(rr + 1) * P], tn[:], ident[:])
                nc.vector.tensor_copy(textT[:F, rg * RB * P:(rg + 1) * RB * P], pt[:])

            # hid[p, u*F + o] = hidden[U*p + u, o] = (text @ W)[U*p+u, o]
            # Stored as fp32r: the PSUM->SBUF copy performs the rounding.
            hid = cpool.tile([P, U * F], f32r)
            textT3 = textT[:F, :].rearrange("f (p u) -> f p u", u=U)
            HG = 8                                   # matmuls per PSUM bank
            for g in range(U // HG):
                ph = pprep.tile([P, HG * F], f32, tag="ph")
                for uu in range(HG):
                    u = g * HG + uu
                    nc.tensor.matmul(
                        ph[:, uu * F:(uu + 1) * F],
                        lhsT=textT3[:, :, u],
                        rhs=w_sb[:F, :],
                        start=True, stop=True,
                    )
                nc.vector.tensor_copy(hid[:, g * HG * F:(g + 1) * HG * F], ph[:])

            hid3 = hid[:].rearrange("p (u f) -> p u f", u=U)

            for rep in range(reps):
                for s in range(nt):
                    x = xpool.tile([P, nsub, U], f32r, tag="x")
                    if x_split:
                        nc.sync.dma_start(x[:, :nsub // 2], adj_d[s, :, :nsub // 2])
                        nc.scalar.dma_start(x[:, nsub // 2:], adj_d[s, :, nsub // 2:])
                    else:
                        nc.sync.dma_start(x[:], adj_d[s])
                    po = pmain.tile([F, nsub], f32, tag="po")
                    for u in range(U):
                        nc.tensor.matmul(
                            po[:],
                            lhsT=hid3[:, u, :],
                            rhs=x[:, :, u],
                            start=(u == 0), stop=(u == U - 1),
                        )
                    ot = spool.tile([F, nsub], f32, tag="ot")
                    nc.vector.tensor_scalar_add(ot[:], po[:], bias_sb[:F, :])
                    getattr(nc, sm_eng_name).dma_start(out_d[:, s * nsub:(s + 1) * nsub], ot[:])

    nc.finalize()
    return nc


def interleave_adj(adj, nsub: int = NSUB):
    """Host-side sharding prep: re-block adj so each core's DMA is fully
    contiguous.  adj [B, L, L] -> [B, L//nsub, P, nsub, U] with
    adj_il[b, s, p, n, u] = adj[b, s*nsub + n, U*p + u]."""
    from concurrent.futures import ThreadPoolExecutor
    nt = L // nsub
    out = np.empty((B, nt, P, nsub, U), dtype=np.float32)
    src = adj.reshape(B, nt, nsub, P, U)

    def one(b):
        np.copyto(out[b], src[b].transpose(0, 2, 1, 3))

    with ThreadPoolExecutor(max_workers=B) as ex:
        list(ex.map(one, range(B)))
    return out


_NC_CACHE = None


def kernel(text, adj, weight, bias):
    global _NC_CACHE
    text = np.ascontiguousarray(np.asarray(text, dtype=np.float32))
    adj = np.asarray(adj, dtype=np.float32)
    weight = np.ascontiguousarray(np.asarray(weight, dtype=np.float32))
    bias = np.ascontiguousarray(np.asarray(bias, dtype=np.float32))
    assert text.shape == (B, L, F) and adj.shape == (B, L, L)

    adj_il = interleave_adj(adj)

    if _NC_CACHE is None:
        _NC_CACHE = build_nc()
    nc = _NC_CACHE

    in_maps = [
        {"text": text[b], "adj_il": adj_il[b], "weight": weight, "bias": bias}
        for b in range(B)
    ]
    last_err = None
    for attempt in range(3):
        try:
            res = run_bass_kernel_spmd(nc, in_maps, list(range(B)))
            break
        except Exception as e:  # transient device wedge (e.g. NRT_EXEC_UNIT_*)
            last_err = e
            import time as _time
            _time.sleep(5 * (attempt + 1))
    else:
        raise last_err
    out = np.stack([res.results[b]["out_t"].T for b in range(B)], axis=0)
    return np.ascontiguousarray(out, dtype=np.float32)
